# revision 25
# baseline (speedup 1.0000x reference)
"""TRN2 Bass kernel for nn_Attention_20633022890922.

The reference module's einsum 'bqhk,bvhd->bqhd' contracts the attention-weight
head axis (k) and the value head axis (v) independently, so the product
factorizes into (sum_k softmax(...)) * (sum_v V) = 1 * Vsum.  The whole module
is therefore algebraically a single linear layer:

    out = tokens @ Wv_sum @ Wo_sum + bo
      Wv_sum[h, d]  = sum_v Wv[h, v*64 + d]          (512 x 64)
      Wo_sum[d, e]  = sum_q Wo[q*64 + d, e]          (64 x 512)

(The only approximation is softmax summing to 1.0, which holds to ~1e-7 in
fp32.)  Wq / Wk cancel entirely.

Device strategy: data-parallel over the batch dim (8 batches -> 8 cores).
Per core: Y = X @ Wv_sum @ Wo_sum with X [8192, 512].

Everything crosses HBM as fp16 (X in: 8 MB, Y out: 8 MB, tiny weights); the
host pre-transposes X to hid-major and un-transposes Y^T afterwards, so every
device DMA descriptor is a long contiguous run (loads 4 KB, stores 2 KB --
small descriptors pay real per-packet overhead on the SDMA engines).  Error
budget: 5.1e-4 max-rel end-to-end vs the 2e-2 gate (the PE quantizes matmul
operands to ~12 mantissa bits anyway, so fp16 is nearly free).

The kernel is DMA-paced (~47us at the 358 GB/s per-core HBM cap vs ~31us of
matmul streaming), so the PE stream must stay dense and warm:
  - GEMM1 of wave w+1 is emitted before GEMM2 of wave w so the PSUM->SBUF
    cast latency hides under matmul streaming;
  - GEMM1 duplicates the T rows (stationary [128h, 64+64 dup] - free, the
    PE streams by moving-operand columns) so GEMM2 can alternate its
    stationary between array row-halves via tile_position; the LDWEIGHTS
    of one half overlaps the in-flight matmul on the other half;
  - garbage "filler" matmuls into just-consumed PSUM keep the PE clock at
    2.4 GHz through the per-wave DMA waits (HAM re-throttles an idle PE
    to 1.2 GHz, which stretched the whole pipeline by ~10%).
GEMM2 computes Y^T (wo blocks stationary, T^T moving), so stores write
[hid, token] rows: 1024 contiguous tokens per partition per wave.
"""

import time

import numpy as np

from concourse import bacc, mybir, tile
from concourse import bass_utils

B, N_TOK, HID, EMB, NH, HD = 8, 8192, 512, 512, 8, 64
N_CORES = 8
CH = 512                      # tokens per compute chunk
WAVE = 1024                   # tokens per compute wave
LWAVE = 2048                  # tokens per load wave (pair of compute waves)
NCHUNK = N_TOK // CH          # 16
NWAVE = N_TOK // WAVE         # 8
CPW = WAVE // CH              # chunks per wave = 2

F32 = mybir.dt.float32
FP16 = mybir.dt.float16

_compiled = None


def _build():
    nc = bacc.Bacc(
        trn_type="TRN2", target_bir_lowering=False, debug=False, num_devices=N_CORES
    )

    # host-transposed fp16 X: [4 hid-blocks, 128 hid, 8192 tokens]
    xf_d = nc.dram_tensor("xf", [4, 128, N_TOK], FP16, kind="ExternalInput")
    # packed consts: [4 x (wv_j | wv_j) stationaries, 512 cols | wo-dup 512]
    cw_d = nc.dram_tensor("cw", [128, 1024], FP16, kind="ExternalInput")
    # transposed output Y^T [hid, tokens]
    y_d = nc.dram_tensor("y", [HID, N_TOK], FP16, kind="ExternalOutput")

    with tile.TileContext(nc) as tc:
        with (
            tc.tile_pool(name="const", bufs=1) as constp,
            tc.tile_pool(name="xt", bufs=4) as xt_p,
            tc.tile_pool(name="tt", bufs=3) as tt_p,
            tc.tile_pool(name="yout", bufs=4) as y_p,
            tc.tile_pool(name="ps_t", bufs=2, space="PSUM") as ps_t,
            tc.tile_pool(name="ps_y", bufs=2, space="PSUM") as ps_y,
        ):
            cw = constp.tile([128, 1024], FP16, tag="cw")
            # split const load: the first GEMM1 matmuls only need wv block 0
            nc.sync.dma_start(cw[:, 0:512], cw_d[:, 0:512])
            nc.scalar.dma_start(cw[:, 512:1024], cw_d[:, 512:1024])

            # ---- all waves resident: loads stream at line rate, the PE
            # never waits on flow control.  Load in 512 KB per-hid-block
            # transfers covering two compute waves (4 KB descriptors); the
            # first pair is split in half so wave 0's blocks land sooner.
            xt_by_pair = []
            for lw in range(N_TOK // LWAVE):
                t = xt_p.tile([128, 4 * LWAVE], FP16, tag="xt", name=f"xt{lw}")
                if lw == 0:
                    for half in range(2):
                        for j in range(4):
                            lo = half * WAVE
                            nc.sync.dma_start(
                                t[:, j * LWAVE + lo:j * LWAVE + lo + WAVE],
                                xf_d[j, :, lo:lo + WAVE],
                            )
                else:
                    for j in range(4):
                        nc.sync.dma_start(
                            t[:, j * LWAVE:(j + 1) * LWAVE],
                            xf_d[j, :, lw * LWAVE:(lw + 1) * LWAVE],
                        )
                xt_by_pair.append(t)

            def xslice(w, j, q):
                t = xt_by_pair[w // 2]
                base = j * LWAVE + (w % 2) * WAVE + q * CH
                return t[:, base:base + CH]

            def gemm1(w):
                # one 2-bank PSUM tile holds the whole wave's T^T
                pts = ps_t.tile([128, CPW, CH], F32, tag="pt", name=f"pt{w}")
                for j in range(4):
                    ws = cw[:, j * 128:(j + 1) * 128]
                    for q in range(CPW):
                        nc.tensor.matmul(
                            pts[:, q, :], ws, xslice(w, j, q),
                            start=(j == 0), stop=(j == 3),
                            skip_group_check=True,
                        )
                return pts

            def gemm2(w, pts):
                # yoT[p, m, t]: Y^T rows m*128+p, wave-token t
                yo = y_p.tile([128, 4, WAVE], FP16, tag="yo", name=f"yo{w}")
                toff = 0
                # single whole-wave cast of T^T (rows duplicated) to SBUF
                # as fp16 for GEMM2; alternate between the two PSUM-capable
                # engines to balance their load
                tt = tt_p.tile([128, CPW * CH], FP16, tag="tt")
                if w % 2 == 0:
                    nc.vector.tensor_copy(tt[:], pts[:])
                else:
                    nc.scalar.copy(tt[:], pts[:])
                for q in range(CPW):
                    c = w * CPW + q
                    for half in range(2):
                        # double-bank PSUM tile: 4 matmuls, one copy
                        py = ps_y.tile([128, 2, CH], F32, tag="py")
                        for s in range(2):
                            m = half * 2 + s
                            # GEMM2 (K=64): Y^T block = wo_m^T @ T^T, as 2
                            # M=64 matmuls on disjoint 64x64 array quadrants
                            # (rows h, output cols ch); all 4 matmuls of the
                            # group run concurrently on the PE's subarrays
                            h = (m % 2) * 64
                            for ch in range(2):
                                nc.tensor.matmul(
                                    py[ch * 64:(ch + 1) * 64, s, :],
                                    cw[h:h + 64,
                                       512 + m * 128 + ch * 64:
                                       512 + m * 128 + (ch + 1) * 64],
                                    tt[h:h + 64, q * CH:(q + 1) * CH],
                                    start=True, stop=True,
                                    skip_group_check=True,
                                    tile_position=(h, ch * 64),
                                )
                        # PSUM->SBUF fp32->fp16 copy (1024 elems/partition),
                        # balanced across the two engines with a PSUM port
                        dst = yo[:, 2 * half:2 * half + 2,
                                 toff + q * CH:toff + (q + 1) * CH]
                        if (c + half) % 2 == 0:
                            nc.vector.tensor_copy(dst, py[:])
                        else:
                            nc.scalar.copy(dst, py[:])

                # stores issue from the sync engine: its queue is idle once
                # the loads are in flight, and nothing the PE depends on
                # ever queues behind a bulky store-issue op (the scalar
                # queue carries casts/copies the PE waits on)
                ydst = y_d[:, w * WAVE:(w + 1) * WAVE].rearrange(
                    "(m p) t -> p m t", p=128
                )
                if w < NWAVE - 1:
                    nc.sync.dma_start(ydst, yo[:])
                else:
                    # final wave: 4 small stores on both rings so the
                    # last completion receipt is short and parallel
                    for m in range(4):
                        eng = nc.sync if m % 2 == 0 else nc.scalar
                        eng.dma_start(ydst[:, m, :], yo[:, m, :])

            # ---- software-pipelined waves: GEMM1(w+1) is emitted (and so
            # runs on the PE) before GEMM2(w), hiding the cast latency
            pts_prev = gemm1(0)
            for w in range(1, NWAVE):
                pts_cur = gemm1(w)
                gemm2(w - 1, pts_prev)
                if w >= 5:
                    # ---- tail HAM fillers: once the loads are done the
                    # compute convoy has unavoidable PE gaps, and an idle
                    # PE re-throttles to 1.2 GHz; garbage matmuls into the
                    # just-consumed PSUM tile keep the clock warm
                    for f in range(4):
                        nc.tensor.matmul(
                            pts_prev[:, 0, :], cw[:, 0:128], cw[:, 0:512],
                            start=True, stop=True, skip_group_check=True,
                        )
                pts_prev = pts_cur
            gemm2(NWAVE - 1, pts_prev)

    nc.compile()
    return nc


def _get_compiled():
    global _compiled
    if _compiled is None:
        _compiled = _build()
    return _compiled


def kernel(tokens, Wq, Wk, Wv, Wo, bo, _trace=False):
    tokens = np.asarray(tokens, dtype=np.float32)
    Wv = np.asarray(Wv, dtype=np.float32)
    Wo = np.asarray(Wo, dtype=np.float32)
    bo = np.asarray(bo, dtype=np.float32)

    # Host-side prep: fold weights, cast everything to fp16, pre-transpose X
    # to hid-major so all device DMAs are contiguous.
    wv_sum = Wv.reshape(HID, NH, HD).sum(axis=1).astype(np.float32)
    wo_sum = Wo.reshape(NH, HD, HID).sum(axis=0).astype(np.float32)
    wv16 = wv_sum.astype(np.float16)
    wo16 = wo_sum.astype(np.float16)
    cw = np.zeros((128, 1024), dtype=np.float16)
    for j in range(4):
        blk = wv16[j * 128:(j + 1) * 128, :]          # [128, 64]
        cw[:, j * 128:j * 128 + 64] = blk
        cw[:, j * 128 + 64:(j + 1) * 128] = blk       # duplicated T rows
    cw[0:64, 512:1024] = wo16
    cw[64:128, 512:1024] = wo16                       # wo on both row-halves

    xf = tokens.astype(np.float16)           # [B, N, 512]
    # -> [B, 4 hid-blocks, 128 hid, N tokens] (host-side transpose)
    xf = np.ascontiguousarray(xf.reshape(B, N_TOK, 4, 128).transpose(0, 2, 3, 1))

    nc = _get_compiled()
    in_maps = [
        {"xf": xf[b], "cw": cw}
        for b in range(N_CORES)
    ]
    # retry once or twice on transient device flakes (rare NRT_EXEC_UNIT
    # wedges have been observed under the axon PJRT path)
    for attempt in range(3):
        try:
            res = bass_utils.run_bass_kernel_spmd(
                nc, in_maps, core_ids=list(range(N_CORES)), trace=_trace
            )
            break
        except Exception:
            if attempt == 2:
                raise
            time.sleep(20)
    # device returns Y^T [hid, tokens]; un-transpose and cast up
    out = np.stack(
        [res.results[b]["y"].T for b in range(N_CORES)], axis=0
    ).astype(np.float32)
    if np.any(bo):
        out += bo
    if _trace:
        return out, res
    return out


if __name__ == "__main__":
    rng = np.random.default_rng(0)
    ins = {
        "tokens": rng.standard_normal((B, N_TOK, HID)).astype(np.float32),
        "Wq": (rng.standard_normal((HID, EMB)) * 0.02).astype(np.float32),
        "Wk": (rng.standard_normal((HID, EMB)) * 0.02).astype(np.float32),
        "Wv": (rng.standard_normal((HID, HID)) * 0.02).astype(np.float32),
        "Wo": (rng.standard_normal((EMB, HID)) * 0.02).astype(np.float32),
        "bo": np.zeros((HID,), dtype=np.float32),
    }
    out = kernel(**ins)
    print(out.shape, out.dtype)
